# revision 14
# baseline (speedup 1.0000x reference)
"""DenoisingPointNet (kNN + gather + 3-layer MLP w/ train-mode BN + maxpool) on 8 TRN2 cores.

Sharding: queries (points) split across the 8 cores (2048 per batch per core);
keys (the full point set) replicated. Each core computes exact top-16 neighbors
for its queries against all 16384 keys (bf16 3-way-split matmul => f32-accurate
distances on the PE, per-1024-segment max8/max_index top-k on the DVE), gathers
features (GPSIMD ap_gather from a packed fp16-pair table, both batches at once),
runs the MLP in fp16 with globally-synchronized batchnorm stats (AllReduce),
maxpools over the 16 neighbors, and writes its slice of the output.
"""
import os
import numpy as np
import ml_dtypes
import concourse.tile as tile
import concourse.mybir as mybir
from concourse import bacc
from concourse.bass_utils import run_bass_kernel_spmd
from concourse.alu_op_type import AluOpType
from contextlib import ExitStack

B = 2
N = 16384
C = 64
KNN = 16
O = 128
NCORES = 8
QPC = N // NCORES          # 2048 queries per batch per core
CHUNKS_PB = QPC // 128     # 16 chunks of 128 queries per batch
ROWS_PB = QPC * KNN        # 32768 MLP rows per batch per core
SEG = 1024
NSEG = N // SEG            # 16
EPS = 1e-5
NT_GLOBAL = float(B * N * KNN)

LAST_EXEC_NS = None


def _split3(x):
    hi = x.astype(ml_dtypes.bfloat16).astype(np.float32)
    r = (x - hi).astype(np.float32)
    mid = r.astype(ml_dtypes.bfloat16).astype(np.float32)
    lo = (r - mid).astype(np.float32)
    return hi, mid, lo


def _build_distance_operands(xyz, sq):
    """bf16 3-way-split operands for s = -d = 2q.k - |q|^2 - |k|^2."""
    Ls = np.zeros((B, NCORES, CHUNKS_PB, 24, 128), np.float32)
    Rs = np.zeros((B, 24, N), np.float32)
    for b in range(B):
        kh, km, kl = _split3(xyz[b].T)
        nkh, nkm, nkl = _split3(-sq[b][None])
        oN = np.ones((N,), np.float32)
        qh_a, qm_a, ql_a = _split3((2.0 * xyz[b]).T)
        nqh_a, nqm_a, nql_a = _split3(-sq[b][None])
        rows_R = []
        for c in range(3): rows_R.append(kh[c])
        rows_R.append(oN); rows_R.append(nkh[0])
        for c in range(3): rows_R.append(km[c])
        for c in range(3): rows_R.append(kh[c])
        rows_R.append(oN); rows_R.append(nkm[0])
        for c in range(3): rows_R.append(km[c])
        for c in range(3): rows_R.append(kl[c])
        for c in range(3): rows_R.append(kh[c])
        rows_R.append(oN); rows_R.append(nkl[0])
        Rs[b] = np.stack(rows_R)
        for core in range(NCORES):
            for ch in range(CHUNKS_PB):
                s0 = core * QPC + ch * 128
                sl = slice(s0, s0 + 128)
                qh, qm, ql = qh_a[:, sl], qm_a[:, sl], ql_a[:, sl]
                nqh = nqh_a[:, sl]; nqm = nqm_a[:, sl]; nql = nql_a[:, sl]
                o128 = np.ones((128,), np.float32)
                rows_L = []
                for c in range(3): rows_L.append(qh[c])
                rows_L.append(nqh[0]); rows_L.append(o128)
                for c in range(3): rows_L.append(qh[c])
                for c in range(3): rows_L.append(qm[c])
                rows_L.append(nqm[0]); rows_L.append(o128)
                for c in range(3): rows_L.append(qm[c])
                for c in range(3): rows_L.append(qh[c])
                for c in range(3): rows_L.append(ql[c])
                rows_L.append(nql[0]); rows_L.append(o128)
                Ls[b, core, ch] = np.stack(rows_L)
    return (Ls.astype(ml_dtypes.bfloat16), Rs.astype(ml_dtypes.bfloat16))


def _build_program():
    nc = bacc.Bacc("TRN2", target_bir_lowering=False, debug=False, num_devices=NCORES)
    f16 = mybir.dt.float16
    f32 = mybir.dt.float32
    bf16 = mybir.dt.bfloat16
    u32 = mybir.dt.uint32
    i16 = mybir.dt.int16
    AF = mybir.ActivationFunctionType

    t_L = nc.dram_tensor("Lq", [B, CHUNKS_PB, 24, 128], bf16, kind="ExternalInput")
    t_R = nc.dram_tensor("Rk", [B, 24, N], bf16, kind="ExternalInput")
    t_FT = nc.dram_tensor("FTpk", [128, N], f32, kind="ExternalInput")  # packed (b0|b1), dup halves
    t_pcT = nc.dram_tensor("pcT", [B, 3, ROWS_PB], f16, kind="ExternalInput")
    t_W1f = nc.dram_tensor("W1f", [128, 128], f16, kind="ExternalInput")  # dup halves
    t_W1p = nc.dram_tensor("W1p", [3, 128], f16, kind="ExternalInput")
    t_W2 = nc.dram_tensor("W2", [128, 128], f16, kind="ExternalInput")
    t_W3 = nc.dram_tensor("W3", [128, 128], f16, kind="ExternalInput")
    t_bn = nc.dram_tensor("bn", [128, 8], f32, kind="ExternalInput")
    t_out = nc.dram_tensor("out", [B, 128, QPC], f32, kind="ExternalOutput")

    d_Z1 = nc.dram_tensor("Z1sp", [B, 128, ROWS_PB], f16)
    d_Z2 = nc.dram_tensor("Z2sp", [B, 128, ROWS_PB], f16)
    d_st1i = nc.dram_tensor("st1i", [128, 2], f32)
    d_st1o = nc.dram_tensor("st1o", [128, 2], f32, addr_space="Shared")
    d_st2i = nc.dram_tensor("st2i", [128, 2], f32)
    d_st2o = nc.dram_tensor("st2o", [128, 2], f32, addr_space="Shared")

    with tile.TileContext(nc) as tc, ExitStack() as ctx:
        const = ctx.enter_context(tc.tile_pool(name="const", bufs=1))
        stg = ctx.enter_context(tc.tile_pool(name="stg", bufs=6))
        knn_s = ctx.enter_context(tc.tile_pool(name="knn", bufs=3))
        gat = ctx.enter_context(tc.tile_pool(name="gat", bufs=2))
        mlp = ctx.enter_context(tc.tile_pool(name="mlp", bufs=3))
        wide = ctx.enter_context(tc.tile_pool(name="wide", bufs=1))
        psum = ctx.enter_context(tc.tile_pool(name="ps", bufs=3, space="PSUM"))
        psmm = ctx.enter_context(tc.tile_pool(name="psmm", bufs=2, space="PSUM"))

        Rt = const.tile([64, N], bf16, tag="R")
        for b in range(B):
            nc.sync.dma_start(Rt[32 * b:32 * b + 24, :], t_R.ap()[b])
        FT = const.tile([128, N], f32, tag="FT")
        nc.sync.dma_start(FT[:], t_FT.ap())
        W1f = const.tile([128, 128], f16, tag="W1f")
        nc.sync.dma_start(W1f[:], t_W1f.ap())
        W1p = const.tile([3, 128], f16, tag="W1p")
        nc.sync.dma_start(W1p[:], t_W1p.ap())
        W2 = const.tile([128, 128], f16, tag="W2")
        nc.sync.dma_start(W2[:], t_W2.ap())
        W3 = const.tile([128, 128], f16, tag="W3")
        nc.sync.dma_start(W3[:], t_W3.ap())
        bnp = const.tile([128, 8], f32, tag="bnp")
        nc.sync.dma_start(bnp[:], t_bn.ap())

        NC8 = NSEG * 8  # 128 candidates per chunk
        iota_seg = const.tile([128, NC8], u32, tag="iseg")
        nc.gpsimd.iota(iota_seg[:], pattern=[[SEG, NSEG], [0, 8]], base=0, channel_multiplier=0)
        iota128 = const.tile([128, NC8], f32, tag="i128")
        nc.gpsimd.iota(iota128[:], pattern=[[1, NC8]], base=0, channel_multiplier=0,
                       allow_small_or_imprecise_dtypes=True)

        # wrapped gather indices: batch0 lists on partitions 0-63, batch1 on 64-127
        idxw = wide.tile([128, QPC], i16, tag="idxw")

        NBLK = ROWS_PB // 512  # 64 blocks of 512 rows per batch
        s1sum = wide.tile([128, B * NBLK], f32, tag="s1sum")
        s1sq = wide.tile([128, B * NBLK], f32, tag="s1sq")

        # ---------------- Phase 1: kNN + gather + L1 (streamed per chunk) ----------------
        for K in range(CHUNKS_PB):
            for b in range(B):
                Lc = knn_s.tile([64, 128], bf16, tag="Lc")
                nc.sync.dma_start(Lc[32 * b:32 * b + 24, :], t_L.ap()[b, K])
                cand = knn_s.tile([128, NC8], f32, tag="cand")
                ixg = knn_s.tile([128, NC8], u32, tag="ixg")
                for s in range(NSEG):
                    ps = psum.tile([128, SEG], f32)
                    nc.tensor.matmul(ps[:, 0:512], Lc[32 * b:32 * b + 24, :],
                                     Rt[32 * b:32 * b + 24, s * SEG:s * SEG + 512],
                                     start=True, stop=True)
                    nc.tensor.matmul(ps[:, 512:1024], Lc[32 * b:32 * b + 24, :],
                                     Rt[32 * b:32 * b + 24, s * SEG + 512:(s + 1) * SEG],
                                     start=True, stop=True)
                    seg = stg.tile([128, SEG], f32, tag="seg")
                    nc.scalar.copy(seg[:], ps[:])
                    sl = slice(s * 8, (s + 1) * 8)
                    nc.vector.max(cand[:, sl], seg[:])
                    nc.vector.max_index(ixg[:, sl], cand[:, sl], seg[:])

                ixgg = knn_s.tile([128, NC8], u32, tag="ixgg")
                nc.vector.tensor_tensor(ixgg[:], ixg[:], iota_seg[:], op=AluOpType.add)
                ixf = knn_s.tile([128, NC8], f32, tag="ixf")
                nc.vector.tensor_copy(ixf[:], ixgg[:])

                m1 = knn_s.tile([128, 8], f32, tag="m1")
                nc.vector.max(m1[:], cand[:])
                p1 = knn_s.tile([128, 8], u32, tag="p1")
                nc.vector.max_index(p1[:], m1[:], cand[:])
                cand2 = knn_s.tile([128, NC8], f32, tag="cand2")
                nc.vector.match_replace(cand2[:], m1[:], cand[:], -1e30)
                m2 = knn_s.tile([128, 8], f32, tag="m2")
                nc.vector.max(m2[:], cand2[:])
                p2 = knn_s.tile([128, 8], u32, tag="p2")
                nc.vector.max_index(p2[:], m2[:], cand2[:])

                pf = knn_s.tile([128, 16], f32, tag="pf")
                nc.vector.tensor_copy(pf[:, 0:8], p1[:])
                nc.vector.tensor_copy(pf[:, 8:16], p2[:])

                nbrf = knn_s.tile([128, 16], f32, tag="nbrf")
                scr = knn_s.tile([128, NC8], f32, tag="scr")
                for r in range(16):
                    nc.vector.scalar_tensor_tensor(
                        scr[:], iota128[:], pf[:, r:r + 1], ixf[:],
                        op0=AluOpType.is_equal, op1=AluOpType.mult,
                        accum_out=nbrf[:, r:r + 1])

                # [128 q, 16 j] -> wrapped idx layout via DVE 32x32 block transpose
                nbr_u = knn_s.tile([128, 32], i16, tag="nbru")
                nc.vector.tensor_copy(nbr_u[:, 0:16], nbrf[:])
                nbr_t = knn_s.tile([128, 32], i16, tag="nbrt")
                nc.vector.transpose(nbr_t[:], nbr_u[:])
                # block t holds j-rows for queries 32t..32t+31 at cols q%32
                for g in range(4):
                    for t in range(4):
                        nc.scalar.dma_start(
                            idxw[64 * b + 16 * g:64 * b + 16 * g + 16,
                                 K * 128 + 32 * t:K * 128 + 32 * (t + 1)],
                            nbr_t[32 * t:32 * t + 16, 0:32])

            # dual-batch gather: partitions 0-63 <- batch0 rows, 64-127 <- batch1 rows
            gout = gat.tile([128, 2048], f32, tag="gout")
            nc.gpsimd.ap_gather(gout[:], FT[:], idxw[:, K * 128:(K + 1) * 128],
                                channels=128, num_elems=N, d=1, num_idxs=2048)
            g16 = gout[:].bitcast(f16).rearrange("p (n t) -> p n t", t=2)
            for b in range(B):
                for blk in range(4):
                    r0 = K * 2048 + blk * 512
                    col = b * NBLK + K * 4 + blk
                    pcb = mlp.tile([3, 512], f16, tag="pcb")
                    nc.sync.dma_start(pcb[:], t_pcT.ap()[b, :, r0:r0 + 512])
                    ps2 = psmm.tile([128, 512], f32, tag="mm")
                    nc.tensor.matmul(ps2[:], W1f[64 * b:64 * b + 64, :],
                                     g16[64 * b:64 * b + 64, blk * 512:(blk + 1) * 512, b],
                                     start=True, stop=False)
                    nc.tensor.matmul(ps2[:], W1p[:], pcb[:], start=False, stop=True)
                    z1b = mlp.tile([128, 512], f16, tag="z1b")
                    nc.scalar.activation(z1b[:], ps2[:], AF.Copy,
                                         accum_out=s1sum[:, col:col + 1])
                    sqs = mlp.tile([128, 512], f16, tag="sqs")
                    nc.scalar.activation(sqs[:], ps2[:], AF.Square,
                                         accum_out=s1sq[:, col:col + 1])
                    nc.scalar.dma_start(d_Z1.ap()[b, :, r0:r0 + 512], z1b[:])

        def bn_coeffs(sums_tile, gcol, becol, tagp):
            mean = mlp.tile([128, 1], f32, tag=tagp + "mean")
            nc.scalar.mul(mean[:], sums_tile[:, 0:1], 1.0 / NT_GLOBAL)
            ssn = mlp.tile([128, 1], f32, tag=tagp + "ssn")
            nc.scalar.mul(ssn[:], sums_tile[:, 1:2], 1.0 / NT_GLOBAL)
            nvar = mlp.tile([128, 1], f32, tag=tagp + "nvar")
            nc.vector.scalar_tensor_tensor(nvar[:], mean[:], mean[:], ssn[:],
                                           op0=AluOpType.mult, op1=AluOpType.subtract)
            var = mlp.tile([128, 1], f32, tag=tagp + "var")
            nc.scalar.mul(var[:], nvar[:], -1.0)
            sd = mlp.tile([128, 1], f32, tag=tagp + "sd")
            nc.scalar.activation(sd[:], var[:], AF.Sqrt, bias=bnp[:, 5:6])
            rs = mlp.tile([128, 1], f32, tag=tagp + "rs")
            nc.vector.reciprocal(rs[:], sd[:])
            a = mlp.tile([128, 1], f32, tag=tagp + "a")
            nc.vector.tensor_tensor(a[:], bnp[:, gcol:gcol + 1], rs[:], op=AluOpType.mult)
            negc = mlp.tile([128, 1], f32, tag=tagp + "negc")
            nc.vector.scalar_tensor_tensor(negc[:], mean[:], a[:], bnp[:, becol:becol + 1],
                                           op0=AluOpType.mult, op1=AluOpType.subtract)
            cc = mlp.tile([128, 1], f32, tag=tagp + "c")
            nc.scalar.mul(cc[:], negc[:], -1.0)
            return a, cc

        st1 = mlp.tile([128, 2], f32, tag="st1")
        nc.vector.tensor_reduce(st1[:, 0:1], s1sum[:], axis=mybir.AxisListType.X, op=AluOpType.add)
        nc.vector.tensor_reduce(st1[:, 1:2], s1sq[:], axis=mybir.AxisListType.X, op=AluOpType.add)
        nc.sync.dma_start(d_st1i.ap(), st1[:])
        nc.gpsimd.collective_compute(
            "AllReduce", AluOpType.add, replica_groups=[list(range(NCORES))],
            ins=[d_st1i.ap()], outs=[d_st1o.ap()])
        st1g = mlp.tile([128, 2], f32, tag="st1g")
        nc.sync.dma_start(st1g[:], d_st1o.ap())
        a1, c1 = bn_coeffs(st1g, 0, 1, "l1")

        # ---------------- Phase 3: Y1 -> L2 + stats ----------------
        s2sum = wide.tile([128, B * NBLK], f32, tag="s2sum")
        s2sq = wide.tile([128, B * NBLK], f32, tag="s2sq")
        col = 0
        for b in range(B):
            for blk in range(NBLK):
                sl = slice(blk * 512, (blk + 1) * 512)
                z1i = mlp.tile([128, 512], f16, tag="z1i")
                nc.sync.dma_start(z1i[:], d_Z1.ap()[b, :, sl])
                y1 = mlp.tile([128, 512], f16, tag="y1")
                nc.scalar.activation(y1[:], z1i[:], AF.Relu, bias=c1[:], scale=a1[:])
                ps = psmm.tile([128, 512], f32, tag="mm")
                nc.tensor.matmul(ps[:], W2[:], y1[:], start=True, stop=True)
                z2b = mlp.tile([128, 512], f16, tag="z2b")
                nc.scalar.activation(z2b[:], ps[:], AF.Copy, accum_out=s2sum[:, col:col + 1])
                sq2 = mlp.tile([128, 512], f16, tag="sq2")
                nc.vector.scalar_tensor_tensor(sq2[:], z2b[:], 1.0, z2b[:],
                                               op0=AluOpType.mult, op1=AluOpType.mult,
                                               accum_out=s2sq[:, col:col + 1])
                nc.scalar.dma_start(d_Z2.ap()[b, :, sl], z2b[:])
                col += 1

        st2 = mlp.tile([128, 2], f32, tag="st2")
        nc.vector.tensor_reduce(st2[:, 0:1], s2sum[:], axis=mybir.AxisListType.X, op=AluOpType.add)
        nc.vector.tensor_reduce(st2[:, 1:2], s2sq[:], axis=mybir.AxisListType.X, op=AluOpType.add)
        nc.sync.dma_start(d_st2i.ap(), st2[:])
        nc.gpsimd.collective_compute(
            "AllReduce", AluOpType.add, replica_groups=[list(range(NCORES))],
            ins=[d_st2i.ap()], outs=[d_st2o.ap()])
        st2g = mlp.tile([128, 2], f32, tag="st2g")
        nc.sync.dma_start(st2g[:], d_st2o.ap())
        a2, c2 = bn_coeffs(st2g, 2, 3, "l2")

        # ---------------- Phase 4: Y2 -> L3 -> maxpool -> out ----------------
        for b in range(B):
            pooled = wide.tile([128, QPC], f32, tag="pooled")
            for blk in range(NBLK):
                sl = slice(blk * 512, (blk + 1) * 512)
                z2i = mlp.tile([128, 512], f16, tag="z2i")
                nc.sync.dma_start(z2i[:], d_Z2.ap()[b, :, sl])
                y2 = mlp.tile([128, 512], f16, tag="y2")
                nc.scalar.activation(y2[:], z2i[:], AF.Relu, bias=c2[:], scale=a2[:])
                ps = psmm.tile([128, 512], f32, tag="mm")
                nc.tensor.matmul(ps[:], W3[:], y2[:], start=True, stop=True)
                q0 = blk * (512 // KNN)
                nc.vector.tensor_reduce(pooled[:, q0:q0 + 512 // KNN],
                                        ps[:].rearrange("p (a b) -> p a b", b=KNN),
                                        axis=mybir.AxisListType.X, op=AluOpType.max)
            outb = wide.tile([128, QPC], f32, tag="outb")
            nc.scalar.activation(outb[:], pooled[:], AF.Identity, bias=bnp[:, 4:5], scale=1.0)
            nc.sync.dma_start(t_out.ap()[b], outb[:])

    nc.compile()
    return nc


_PROGRAM_CACHE = None


def kernel(features, position_condition, W1, b1, g1, be1, W2, b2, g2, be2, W3, b3):
    global LAST_EXEC_NS, _PROGRAM_CACHE
    features = np.asarray(features, np.float32)
    pc = np.asarray(position_condition, np.float32)

    xyz = pc.mean(axis=2).astype(np.float32)
    sq = (xyz * xyz).sum(-1).astype(np.float32)
    Ls, Rs = _build_distance_operands(xyz, sq)

    f0 = features[0].T.astype(np.float16).view(np.uint16).astype(np.uint32)
    f1 = features[1].T.astype(np.float16).view(np.uint16).astype(np.uint32)
    pk = (f0 | (f1 << 16)).view(np.float32)      # [64, N]
    FTpk = np.concatenate([pk, pk], axis=0)      # [128, N] duplicated halves

    W1n = np.asarray(W1, np.float32)
    W1f = np.zeros((128, 128), np.float16)
    W1f[0:64] = W1n[0:64].astype(np.float16)
    W1f[64:128] = W1n[0:64].astype(np.float16)
    W1p = np.ascontiguousarray(W1n[64:67]).astype(np.float16)
    W2n = np.ascontiguousarray(np.asarray(W2, np.float32)).astype(np.float16)
    W3n = np.ascontiguousarray(np.asarray(W3, np.float32)).astype(np.float16)
    bn = np.zeros((128, 8), np.float32)
    bn[:, 0] = np.asarray(g1, np.float32)
    bn[:, 1] = np.asarray(be1, np.float32)
    bn[:, 2] = np.asarray(g2, np.float32)
    bn[:, 3] = np.asarray(be2, np.float32)
    bn[:, 4] = np.asarray(b3, np.float32)
    bn[:, 5] = EPS

    if _PROGRAM_CACHE is None:
        _PROGRAM_CACHE = _build_program()
    nc = _PROGRAM_CACHE

    in_maps = []
    for core in range(NCORES):
        q0 = core * QPC
        pcT = np.zeros((B, 3, ROWS_PB), np.float16)
        for b in range(B):
            pcT[b] = pc[b, q0:q0 + QPC].reshape(ROWS_PB, 3).T.astype(np.float16)
        in_maps.append({
            "Lq": np.ascontiguousarray(Ls[:, core]),
            "Rk": Rs,
            "FTpk": FTpk,
            "pcT": pcT,
            "W1f": W1f,
            "W1p": W1p,
            "W2": W2n,
            "W3": W3n,
            "bn": bn,
        })

    trace = os.environ.get("DENOISE_TRACE", "0") == "1"
    res = run_bass_kernel_spmd(nc, in_maps, list(range(NCORES)), trace=trace)
    LAST_EXEC_NS = res.exec_time_ns

    out = np.zeros((B, N, O), np.float32)
    for core in range(NCORES):
        o = res.results[core]["out"]
        for b in range(B):
            out[b, core * QPC:(core + 1) * QPC] = o[b].T
    return out


# revision 16
# speedup vs baseline: 1.0837x; 1.0837x over previous
"""DenoisingPointNet (kNN + gather + 3-layer MLP w/ train-mode BN + maxpool) on 8 TRN2 cores.

Sharding: queries (points) split across the 8 cores (2048 per batch per core);
keys (the full point set) replicated. Each core computes exact top-16 neighbors
for its queries against all 16384 keys (bf16 3-way-split matmul => f32-accurate
distances on the PE, per-1024-segment max8/max_index top-k on the DVE), gathers
features (GPSIMD ap_gather from a packed fp16-pair table, both batches at once),
runs the MLP in fp16 with globally-synchronized batchnorm stats (AllReduce),
maxpools over the 16 neighbors, and writes its slice of the output.
"""
import os
import numpy as np
import ml_dtypes
import concourse.tile as tile
import concourse.mybir as mybir
from concourse import bacc
from concourse.bass_utils import run_bass_kernel_spmd
from concourse.alu_op_type import AluOpType
from contextlib import ExitStack

B = 2
N = 16384
C = 64
KNN = 16
O = 128
NCORES = 8
QPC = N // NCORES          # 2048 queries per batch per core
CHUNKS_PB = QPC // 128     # 16 chunks of 128 queries per batch
ROWS_PB = QPC * KNN        # 32768 MLP rows per batch per core
SEG = 1024
NSEG = N // SEG            # 16
EPS = 1e-5
NT_GLOBAL = float(B * N * KNN)

LAST_EXEC_NS = None


def _split3(x):
    hi = x.astype(ml_dtypes.bfloat16).astype(np.float32)
    r = (x - hi).astype(np.float32)
    mid = r.astype(ml_dtypes.bfloat16).astype(np.float32)
    lo = (r - mid).astype(np.float32)
    return hi, mid, lo


def _build_distance_operands(xyz, sq):
    """bf16 3-way-split operands for s = -d = 2q.k - |q|^2 - |k|^2."""
    Ls = np.zeros((B, NCORES, CHUNKS_PB, 24, 128), np.float32)
    Rs = np.zeros((B, 24, N), np.float32)
    for b in range(B):
        kh, km, kl = _split3(xyz[b].T)
        nkh, nkm, nkl = _split3(-sq[b][None])
        oN = np.ones((N,), np.float32)
        qh_a, qm_a, ql_a = _split3((2.0 * xyz[b]).T)
        nqh_a, nqm_a, nql_a = _split3(-sq[b][None])
        rows_R = []
        for c in range(3): rows_R.append(kh[c])
        rows_R.append(oN); rows_R.append(nkh[0])
        for c in range(3): rows_R.append(km[c])
        for c in range(3): rows_R.append(kh[c])
        rows_R.append(oN); rows_R.append(nkm[0])
        for c in range(3): rows_R.append(km[c])
        for c in range(3): rows_R.append(kl[c])
        for c in range(3): rows_R.append(kh[c])
        rows_R.append(oN); rows_R.append(nkl[0])
        Rs[b] = np.stack(rows_R)
        for core in range(NCORES):
            for ch in range(CHUNKS_PB):
                s0 = core * QPC + ch * 128
                sl = slice(s0, s0 + 128)
                qh, qm, ql = qh_a[:, sl], qm_a[:, sl], ql_a[:, sl]
                nqh = nqh_a[:, sl]; nqm = nqm_a[:, sl]; nql = nql_a[:, sl]
                o128 = np.ones((128,), np.float32)
                rows_L = []
                for c in range(3): rows_L.append(qh[c])
                rows_L.append(nqh[0]); rows_L.append(o128)
                for c in range(3): rows_L.append(qh[c])
                for c in range(3): rows_L.append(qm[c])
                rows_L.append(nqm[0]); rows_L.append(o128)
                for c in range(3): rows_L.append(qm[c])
                for c in range(3): rows_L.append(qh[c])
                for c in range(3): rows_L.append(ql[c])
                rows_L.append(nql[0]); rows_L.append(o128)
                Ls[b, core, ch] = np.stack(rows_L)
    return (Ls.astype(ml_dtypes.bfloat16), Rs.astype(ml_dtypes.bfloat16))


def _build_program():
    nc = bacc.Bacc("TRN2", target_bir_lowering=False, debug=False, num_devices=NCORES)
    f16 = mybir.dt.float16
    f32 = mybir.dt.float32
    bf16 = mybir.dt.bfloat16
    u32 = mybir.dt.uint32
    i16 = mybir.dt.int16
    AF = mybir.ActivationFunctionType

    t_L = nc.dram_tensor("Lq", [B, CHUNKS_PB, 24, 128], bf16, kind="ExternalInput")
    t_R = nc.dram_tensor("Rk", [B, 24, N], bf16, kind="ExternalInput")
    t_FT = nc.dram_tensor("FTpk", [128, N], f32, kind="ExternalInput")  # packed (b0|b1), dup halves
    t_pcT = nc.dram_tensor("pcT", [B, 3, ROWS_PB], f16, kind="ExternalInput")
    t_W1f = nc.dram_tensor("W1f", [128, 128], f16, kind="ExternalInput")  # dup halves
    t_W1p = nc.dram_tensor("W1p", [3, 128], f16, kind="ExternalInput")
    t_W2 = nc.dram_tensor("W2", [128, 128], f16, kind="ExternalInput")
    t_W3 = nc.dram_tensor("W3", [128, 128], f16, kind="ExternalInput")
    t_bn = nc.dram_tensor("bn", [128, 8], f32, kind="ExternalInput")
    t_out = nc.dram_tensor("out", [B, 128, QPC], f32, kind="ExternalOutput")

    d_Z1 = nc.dram_tensor("Z1sp", [B, 128, ROWS_PB], f16)
    d_Z2 = nc.dram_tensor("Z2sp", [B, 128, ROWS_PB], f16)
    d_st1i = nc.dram_tensor("st1i", [128, 2], f32)
    d_st1o = nc.dram_tensor("st1o", [128, 2], f32, addr_space="Shared")
    d_st2i = nc.dram_tensor("st2i", [128, 2], f32)
    d_st2o = nc.dram_tensor("st2o", [128, 2], f32, addr_space="Shared")

    with tile.TileContext(nc) as tc, ExitStack() as ctx:
        const = ctx.enter_context(tc.tile_pool(name="const", bufs=1))
        stg = ctx.enter_context(tc.tile_pool(name="stg", bufs=6))
        knn_s = ctx.enter_context(tc.tile_pool(name="knn", bufs=3))
        gat = ctx.enter_context(tc.tile_pool(name="gat", bufs=3))
        mlp = ctx.enter_context(tc.tile_pool(name="mlp", bufs=3))
        wide = ctx.enter_context(tc.tile_pool(name="wide", bufs=1))
        psum = ctx.enter_context(tc.tile_pool(name="ps", bufs=3, space="PSUM"))
        psmm = ctx.enter_context(tc.tile_pool(name="psmm", bufs=2, space="PSUM"))

        Rt = const.tile([64, N], bf16, tag="R")
        for b in range(B):
            nc.sync.dma_start(Rt[32 * b:32 * b + 24, :], t_R.ap()[b])
        FT = const.tile([128, N], f32, tag="FT")
        nc.sync.dma_start(FT[:], t_FT.ap())
        W1f = const.tile([128, 128], f16, tag="W1f")
        nc.sync.dma_start(W1f[:], t_W1f.ap())
        W1p = const.tile([3, 128], f16, tag="W1p")
        nc.sync.dma_start(W1p[:], t_W1p.ap())
        W2 = const.tile([128, 128], f16, tag="W2")
        nc.sync.dma_start(W2[:], t_W2.ap())
        W3 = const.tile([128, 128], f16, tag="W3")
        nc.sync.dma_start(W3[:], t_W3.ap())
        bnp = const.tile([128, 8], f32, tag="bnp")
        nc.sync.dma_start(bnp[:], t_bn.ap())

        NC8 = NSEG * 8  # 128 candidates per chunk
        iota_seg = const.tile([128, NC8], u32, tag="iseg")
        nc.gpsimd.iota(iota_seg[:], pattern=[[SEG, NSEG], [0, 8]], base=0, channel_multiplier=0)
        iota128 = const.tile([128, NC8], f32, tag="i128")
        nc.gpsimd.iota(iota128[:], pattern=[[1, NC8]], base=0, channel_multiplier=0,
                       allow_small_or_imprecise_dtypes=True)

        NBLK = ROWS_PB // 512  # 64 blocks of 512 rows per batch
        s1sum = wide.tile([128, B * NBLK], f32, tag="s1sum")
        s1sq = wide.tile([128, B * NBLK], f32, tag="s1sq")

        # ---------------- Phase 1: kNN + gather + L1 (streamed per chunk) ----------------
        for K in range(CHUNKS_PB):
            idxc = gat.tile([128, 128], i16, tag="idxc")
            for b in range(B):
                Lc = knn_s.tile([64, 128], bf16, tag="Lc")
                nc.sync.dma_start(Lc[32 * b:32 * b + 24, :], t_L.ap()[b, K])
                cand = knn_s.tile([128, NC8], f32, tag="cand")
                ixg = knn_s.tile([128, NC8], u32, tag="ixg")
                for s in range(NSEG):
                    ps = psum.tile([128, SEG], f32)
                    nc.tensor.matmul(ps[:, 0:512], Lc[32 * b:32 * b + 24, :],
                                     Rt[32 * b:32 * b + 24, s * SEG:s * SEG + 512],
                                     start=True, stop=True)
                    nc.tensor.matmul(ps[:, 512:1024], Lc[32 * b:32 * b + 24, :],
                                     Rt[32 * b:32 * b + 24, s * SEG + 512:(s + 1) * SEG],
                                     start=True, stop=True)
                    seg = stg.tile([128, SEG], f32, tag="seg")
                    nc.scalar.copy(seg[:], ps[:])
                    sl = slice(s * 8, (s + 1) * 8)
                    nc.vector.max(cand[:, sl], seg[:])
                    nc.vector.max_index(ixg[:, sl], cand[:, sl], seg[:])

                ixgg = knn_s.tile([128, NC8], u32, tag="ixgg")
                nc.vector.tensor_tensor(ixgg[:], ixg[:], iota_seg[:], op=AluOpType.add)
                ixf = knn_s.tile([128, NC8], f32, tag="ixf")
                nc.vector.tensor_copy(ixf[:], ixgg[:])

                m1 = knn_s.tile([128, 8], f32, tag="m1")
                nc.vector.max(m1[:], cand[:])
                p1 = knn_s.tile([128, 8], u32, tag="p1")
                nc.vector.max_index(p1[:], m1[:], cand[:])
                cand2 = knn_s.tile([128, NC8], f32, tag="cand2")
                nc.vector.match_replace(cand2[:], m1[:], cand[:], -1e30)
                m2 = knn_s.tile([128, 8], f32, tag="m2")
                nc.vector.max(m2[:], cand2[:])
                p2 = knn_s.tile([128, 8], u32, tag="p2")
                nc.vector.max_index(p2[:], m2[:], cand2[:])

                pf = knn_s.tile([128, 16], f32, tag="pf")
                nc.vector.tensor_copy(pf[:, 0:8], p1[:])
                nc.vector.tensor_copy(pf[:, 8:16], p2[:])

                nbrf = knn_s.tile([128, 16], f32, tag="nbrf")
                scr = knn_s.tile([128, NC8], f32, tag="scr")
                for r in range(16):
                    nc.vector.scalar_tensor_tensor(
                        scr[:], iota128[:], pf[:, r:r + 1], ixf[:],
                        op0=AluOpType.is_equal, op1=AluOpType.mult,
                        accum_out=nbrf[:, r:r + 1])

                # [128 q, 16 j] -> wrapped idx layout via DVE 32x32 block transpose
                nbr_u = knn_s.tile([128, 32], i16, tag="nbru")
                nc.vector.tensor_copy(nbr_u[:, 0:16], nbrf[:])
                nbr_t = knn_s.tile([128, 32], i16, tag="nbrt")
                nc.vector.transpose(nbr_t[:], nbr_u[:])
                # block t holds j-rows for queries 32t..32t+31 at cols q%32
                for g in range(4):
                    for t in range(4):
                        nc.sync.dma_start(
                            idxc[64 * b + 16 * g:64 * b + 16 * g + 16,
                                 32 * t:32 * (t + 1)],
                            nbr_t[32 * t:32 * t + 16, 0:32])

            # dual-batch gather: partitions 0-63 <- batch0 rows, 64-127 <- batch1 rows
            gout = gat.tile([128, 2048], f32, tag="gout")
            nc.gpsimd.ap_gather(gout[:], FT[:], idxc[:],
                                channels=128, num_elems=N, d=1, num_idxs=2048)
            g16 = gout[:].bitcast(f16).rearrange("p (n t) -> p n t", t=2)
            for b in range(B):
                for blk in range(4):
                    r0 = K * 2048 + blk * 512
                    col = b * NBLK + K * 4 + blk
                    pcb = mlp.tile([3, 512], f16, tag="pcb")
                    nc.sync.dma_start(pcb[:], t_pcT.ap()[b, :, r0:r0 + 512])
                    ps2 = psmm.tile([128, 512], f32, tag="mm")
                    nc.tensor.matmul(ps2[:], W1f[64 * b:64 * b + 64, :],
                                     g16[64 * b:64 * b + 64, blk * 512:(blk + 1) * 512, b],
                                     start=True, stop=False)
                    nc.tensor.matmul(ps2[:], W1p[:], pcb[:], start=False, stop=True)
                    z1b = mlp.tile([128, 512], f16, tag="z1b")
                    nc.scalar.activation(z1b[:], ps2[:], AF.Copy,
                                         accum_out=s1sum[:, col:col + 1])
                    sqs = mlp.tile([128, 512], f16, tag="sqs")
                    nc.scalar.activation(sqs[:], ps2[:], AF.Square,
                                         accum_out=s1sq[:, col:col + 1])
                    nc.sync.dma_start(d_Z1.ap()[b, :, r0:r0 + 512], z1b[:])

        def bn_coeffs(sums_tile, gcol, becol, tagp):
            mean = mlp.tile([128, 1], f32, tag=tagp + "mean")
            nc.scalar.mul(mean[:], sums_tile[:, 0:1], 1.0 / NT_GLOBAL)
            ssn = mlp.tile([128, 1], f32, tag=tagp + "ssn")
            nc.scalar.mul(ssn[:], sums_tile[:, 1:2], 1.0 / NT_GLOBAL)
            nvar = mlp.tile([128, 1], f32, tag=tagp + "nvar")
            nc.vector.scalar_tensor_tensor(nvar[:], mean[:], mean[:], ssn[:],
                                           op0=AluOpType.mult, op1=AluOpType.subtract)
            var = mlp.tile([128, 1], f32, tag=tagp + "var")
            nc.scalar.mul(var[:], nvar[:], -1.0)
            sd = mlp.tile([128, 1], f32, tag=tagp + "sd")
            nc.scalar.activation(sd[:], var[:], AF.Sqrt, bias=bnp[:, 5:6])
            rs = mlp.tile([128, 1], f32, tag=tagp + "rs")
            nc.vector.reciprocal(rs[:], sd[:])
            a = mlp.tile([128, 1], f32, tag=tagp + "a")
            nc.vector.tensor_tensor(a[:], bnp[:, gcol:gcol + 1], rs[:], op=AluOpType.mult)
            negc = mlp.tile([128, 1], f32, tag=tagp + "negc")
            nc.vector.scalar_tensor_tensor(negc[:], mean[:], a[:], bnp[:, becol:becol + 1],
                                           op0=AluOpType.mult, op1=AluOpType.subtract)
            cc = mlp.tile([128, 1], f32, tag=tagp + "c")
            nc.scalar.mul(cc[:], negc[:], -1.0)
            return a, cc

        st1 = mlp.tile([128, 2], f32, tag="st1")
        nc.vector.tensor_reduce(st1[:, 0:1], s1sum[:], axis=mybir.AxisListType.X, op=AluOpType.add)
        nc.vector.tensor_reduce(st1[:, 1:2], s1sq[:], axis=mybir.AxisListType.X, op=AluOpType.add)
        nc.sync.dma_start(d_st1i.ap(), st1[:])
        nc.gpsimd.collective_compute(
            "AllReduce", AluOpType.add, replica_groups=[list(range(NCORES))],
            ins=[d_st1i.ap()], outs=[d_st1o.ap()])
        st1g = mlp.tile([128, 2], f32, tag="st1g")
        nc.sync.dma_start(st1g[:], d_st1o.ap())
        a1, c1 = bn_coeffs(st1g, 0, 1, "l1")

        # ---------------- Phase 3: Y1 -> L2 + stats ----------------
        s2sum = wide.tile([128, B * NBLK], f32, tag="s2sum")
        s2sq = wide.tile([128, B * NBLK], f32, tag="s2sq")
        col = 0
        for b in range(B):
            for blk in range(NBLK):
                sl = slice(blk * 512, (blk + 1) * 512)
                z1i = mlp.tile([128, 512], f16, tag="z1i")
                nc.sync.dma_start(z1i[:], d_Z1.ap()[b, :, sl])
                y1 = mlp.tile([128, 512], f16, tag="y1")
                nc.scalar.activation(y1[:], z1i[:], AF.Relu, bias=c1[:], scale=a1[:])
                ps = psmm.tile([128, 512], f32, tag="mm")
                nc.tensor.matmul(ps[:], W2[:], y1[:], start=True, stop=True)
                z2b = mlp.tile([128, 512], f16, tag="z2b")
                nc.scalar.activation(z2b[:], ps[:], AF.Copy, accum_out=s2sum[:, col:col + 1])
                sq2 = mlp.tile([128, 512], f16, tag="sq2")
                nc.vector.scalar_tensor_tensor(sq2[:], z2b[:], 1.0, z2b[:],
                                               op0=AluOpType.mult, op1=AluOpType.mult,
                                               accum_out=s2sq[:, col:col + 1])
                nc.sync.dma_start(d_Z2.ap()[b, :, sl], z2b[:])
                col += 1

        st2 = mlp.tile([128, 2], f32, tag="st2")
        nc.vector.tensor_reduce(st2[:, 0:1], s2sum[:], axis=mybir.AxisListType.X, op=AluOpType.add)
        nc.vector.tensor_reduce(st2[:, 1:2], s2sq[:], axis=mybir.AxisListType.X, op=AluOpType.add)
        nc.sync.dma_start(d_st2i.ap(), st2[:])
        nc.gpsimd.collective_compute(
            "AllReduce", AluOpType.add, replica_groups=[list(range(NCORES))],
            ins=[d_st2i.ap()], outs=[d_st2o.ap()])
        st2g = mlp.tile([128, 2], f32, tag="st2g")
        nc.sync.dma_start(st2g[:], d_st2o.ap())
        a2, c2 = bn_coeffs(st2g, 2, 3, "l2")

        # ---------------- Phase 4: Y2 -> L3 -> maxpool -> out ----------------
        for b in range(B):
            pooled = wide.tile([128, QPC], f32, tag="pooled")
            for blk in range(NBLK):
                sl = slice(blk * 512, (blk + 1) * 512)
                z2i = mlp.tile([128, 512], f16, tag="z2i")
                nc.sync.dma_start(z2i[:], d_Z2.ap()[b, :, sl])
                y2 = mlp.tile([128, 512], f16, tag="y2")
                nc.scalar.activation(y2[:], z2i[:], AF.Relu, bias=c2[:], scale=a2[:])
                ps = psmm.tile([128, 512], f32, tag="mm")
                nc.tensor.matmul(ps[:], W3[:], y2[:], start=True, stop=True)
                q0 = blk * (512 // KNN)
                nc.vector.tensor_reduce(pooled[:, q0:q0 + 512 // KNN],
                                        ps[:].rearrange("p (a b) -> p a b", b=KNN),
                                        axis=mybir.AxisListType.X, op=AluOpType.max)
            outb = wide.tile([128, QPC], f32, tag="outb")
            nc.scalar.activation(outb[:], pooled[:], AF.Identity, bias=bnp[:, 4:5], scale=1.0)
            nc.sync.dma_start(t_out.ap()[b], outb[:])

    nc.compile()
    return nc


_PROGRAM_CACHE = None


def kernel(features, position_condition, W1, b1, g1, be1, W2, b2, g2, be2, W3, b3):
    global LAST_EXEC_NS, _PROGRAM_CACHE
    features = np.asarray(features, np.float32)
    pc = np.asarray(position_condition, np.float32)

    xyz = pc.mean(axis=2).astype(np.float32)
    sq = (xyz * xyz).sum(-1).astype(np.float32)
    Ls, Rs = _build_distance_operands(xyz, sq)

    f0 = features[0].T.astype(np.float16).view(np.uint16).astype(np.uint32)
    f1 = features[1].T.astype(np.float16).view(np.uint16).astype(np.uint32)
    pk = (f0 | (f1 << 16)).view(np.float32)      # [64, N]
    FTpk = np.concatenate([pk, pk], axis=0)      # [128, N] duplicated halves

    W1n = np.asarray(W1, np.float32)
    W1f = np.zeros((128, 128), np.float16)
    W1f[0:64] = W1n[0:64].astype(np.float16)
    W1f[64:128] = W1n[0:64].astype(np.float16)
    W1p = np.ascontiguousarray(W1n[64:67]).astype(np.float16)
    W2n = np.ascontiguousarray(np.asarray(W2, np.float32)).astype(np.float16)
    W3n = np.ascontiguousarray(np.asarray(W3, np.float32)).astype(np.float16)
    bn = np.zeros((128, 8), np.float32)
    bn[:, 0] = np.asarray(g1, np.float32)
    bn[:, 1] = np.asarray(be1, np.float32)
    bn[:, 2] = np.asarray(g2, np.float32)
    bn[:, 3] = np.asarray(be2, np.float32)
    bn[:, 4] = np.asarray(b3, np.float32)
    bn[:, 5] = EPS

    if _PROGRAM_CACHE is None:
        _PROGRAM_CACHE = _build_program()
    nc = _PROGRAM_CACHE

    in_maps = []
    for core in range(NCORES):
        q0 = core * QPC
        pcT = np.zeros((B, 3, ROWS_PB), np.float16)
        for b in range(B):
            pcT[b] = pc[b, q0:q0 + QPC].reshape(ROWS_PB, 3).T.astype(np.float16)
        in_maps.append({
            "Lq": np.ascontiguousarray(Ls[:, core]),
            "Rk": Rs,
            "FTpk": FTpk,
            "pcT": pcT,
            "W1f": W1f,
            "W1p": W1p,
            "W2": W2n,
            "W3": W3n,
            "bn": bn,
        })

    trace = os.environ.get("DENOISE_TRACE", "0") == "1"
    res = run_bass_kernel_spmd(nc, in_maps, list(range(NCORES)), trace=trace)
    LAST_EXEC_NS = res.exec_time_ns

    out = np.zeros((B, N, O), np.float32)
    for core in range(NCORES):
        o = res.results[core]["out"]
        for b in range(B):
            out[b, core * QPC:(core + 1) * QPC] = o[b].T
    return out
